# revision 4
# baseline (speedup 1.0000x reference)
"""Trainium2 Bass kernel for nn_MAB_44057774522768 (Set-Transformer MAB block), v2.

Per batch b (B=8 -> core b), Sq=Sk=1024, D=512, H=8 heads, dh=64:
    Qp = Q @ Wq.T + bq;  Kp = K @ Wk.T + bk;  Vp = K @ Wv.T (bv folded out)
    scores_h = Qp_h @ Kp_h.T / sqrt(D);  A = softmax(scores)
    ctx_h = A_h @ Vp_h;  O1 = Qp + ctx
    out = O1 + bv + relu(O1 @ Wo.T + bo + Wo@bv)

v2 design (vs v1 baseline at ~180us/core):
  * all-bf16 inputs/weights (halves HBM traffic; f32 only in PSUM + final out)
  * scores for a head pair run CONCURRENTLY as K=64 row-tiles (partitions
    0-63 / 64-127) -> 2x on the score matmuls
  * scores output bf16 directly to PSUM (1KB/bank), exp'd in one ACT op per
    (pair, k-tile) of FD=1024 -> ACT (the 73us exp floor) stays saturated
  * q-chunked dataflow (n = 512-col chunk outer loop): ctx accumulators are
    [65, 512] -> whole pipeline fits in 8 PSUM banks with double-buffering,
    and FFN(n=0) + its output DMA overlap n=1's attention
  * softmax denominator via ones-column in Vp (row 64 of ctx PSUM);
    whole-tile fp32 reciprocal (single-partition DVE slices at base partition
    64 silently execute at partition 0 -- never slice-recip row 64 alone),
    then DRAM-bounce + partition-step-0 broadcast DMA back to 64 partitions
  * projections and FFN drip-fed into attention's PE gaps as fillers
"""

import math
import os

import numpy as np

import concourse.bass as bass
import concourse.mybir as mybir
import concourse.tile as tile
from concourse import bacc
from concourse.bass_utils import run_bass_kernel_spmd

B, SQ, SK, D = 8, 1024, 1024, 512
H, DH = 8, 64
N_CORES = 8
KC = D // 128   # 4 contraction chunks of 128 (din)
MT = D // 128   # 4 output-feature tiles of 128 (dout)
NQ = SQ // 512  # 2 q chunks of 512
KT8 = SK // 128 # 8 key-seq tiles of 128

F32 = mybir.dt.float32
BF16 = mybir.dt.bfloat16
ALU = mybir.AluOpType
ACTF = mybir.ActivationFunctionType

_NC = None


def _build():
    nc = bacc.Bacc(None, target_bir_lowering=False, debug=False)

    # inputs host-packed to the exact SBUF layout [partition, kc, free] so
    # each loads in ONE descriptor with 4-8KB partition lines (the 3 HWDGE
    # queues run ~20GB/s on 1KB lines but much faster on big contiguous ones)
    dQT = nc.dram_tensor("QT", [128, KC, SQ], BF16, kind="ExternalInput")
    dKT = nc.dram_tensor("KT", [128, KC, SK], BF16, kind="ExternalInput")
    dWq = nc.dram_tensor("WqT", [128, KC, D], BF16, kind="ExternalInput")
    dWk = nc.dram_tensor("WkT", [128, KC, D], BF16, kind="ExternalInput")
    dWv = nc.dram_tensor("WvT", [128, KC, D], BF16, kind="ExternalInput")
    dWo = nc.dram_tensor("WoT", [128, KC, D], BF16, kind="ExternalInput")
    dBQ = nc.dram_tensor("BQ", [128, MT], F32, kind="ExternalInput")
    dBK = nc.dram_tensor("BK", [128, MT], F32, kind="ExternalInput")
    dBO2 = nc.dram_tensor("BO2", [128, MT], F32, kind="ExternalInput")
    dBV = nc.dram_tensor("BV", [128, MT], F32, kind="ExternalInput")
    dOT = nc.dram_tensor("OT", [D, SQ], F32, kind="ExternalOutput")

    dbg = os.environ.get("KDEBUG", "0") == "1"
    if dbg:
        dDQP = nc.dram_tensor("DQP", [128, MT, SQ], BF16, kind="ExternalOutput")
        dDKP = nc.dram_tensor("DKP", [128, MT, SK], BF16, kind="ExternalOutput")
        dDEX = nc.dram_tensor("DEX", [128, 2, 512], BF16, kind="ExternalOutput")
        dDO1 = nc.dram_tensor("DO1", [128, MT, SQ], BF16, kind="ExternalOutput")
        dDCU = nc.dram_tensor("DCU", [128, 512], F32, kind="ExternalOutput")
        dDRB = nc.dram_tensor("DRB", [128, 512], F32, kind="ExternalOutput")

    scale = 1.0 / math.sqrt(float(D))
    ctx_win = int(os.environ.get("KCTXWIN", "0"))

    with tile.TileContext(nc) as tc:
        with (
            tc.tile_pool(name="persist", bufs=1) as persist,
            tc.tile_pool(name="ppool", bufs=2, space="PSUM") as ppool,   # 1 bank x2
            tc.tile_pool(name="spool", bufs=2, space="PSUM") as spool,   # 2 banks x2
            tc.tile_pool(name="cpool", bufs=1, space="PSUM") as cpool,   # 2 banks
            tc.tile_pool(name="epool", bufs=max(3, ctx_win + 2)) as epool,
            tc.tile_pool(name="zpool", bufs=2) as zpool,
            tc.tile_pool(name="ypool", bufs=2) as ypool,
            tc.tile_pool(name="rpool", bufs=2) as rpool,
            tc.tile_pool(name="rbpool", bufs=2) as rbpool,
            tc.tile_pool(name="dpool", bufs=2, space="DRAM") as dpool,
            tc.tile_pool(name="cnpool", bufs=2) as cnpool,
            tc.tile_pool(name="fpool", bufs=2) as fpool,
            tc.tile_pool(name="outpool", bufs=3) as outpool,
        ):
            # ---- persistent SBUF tensors ----
            qt = persist.tile([128, KC, SQ], BF16)
            kt = persist.tile([128, KC, SK], BF16)
            wq = persist.tile([128, KC, D], BF16)
            wk = persist.tile([128, KC, D], BF16)
            wv = persist.tile([128, KC, D], BF16)
            wo = persist.tile([128, KC, D], BF16)
            bq = persist.tile([128, MT], F32)
            bk = persist.tile([128, MT], F32)
            bo2 = persist.tile([128, MT], F32)
            bv = persist.tile([128, MT], F32)
            qp = persist.tile([128, MT, SQ], BF16)   # projected Q (scores rhs + residual)
            kpb = persist.tile([128, MT, SK], BF16)
            # Vp seq-major [k, kt8, h, dh] + ones column at dh=64 per head
            vpa = persist.tile([128, KT8, H, DH + 1], BF16)
            o1 = persist.tile([128, MT, SQ], BF16)

            # ---- input DMAs: one big descriptor per tensor, spread across
            # the 3 queues by first-need (wq/wk gate the upfront projections,
            # qt/kt gate scores, wv gates ctx, wo is needed only by the FFN)
            # sync (SP) is the slow software queue (~50GB/s) -- biases only.
            # scalar + gpsimd are fast hardware queues (~150-250GB/s): weights
            # first (they gate the upfront projections), then activations.
            nc.scalar.dma_start(out=wq[:, :, :], in_=dWq[:, :, :])
            nc.gpsimd.dma_start(out=wk[:, :, :], in_=dWk[:, :, :])
            nc.scalar.dma_start(out=qt[:, :, :], in_=dQT[:, :, :])
            nc.gpsimd.dma_start(out=kt[:, :, :], in_=dKT[:, :, :])
            nc.scalar.dma_start(out=wv[:, :, :], in_=dWv[:, :, :])
            nc.gpsimd.dma_start(out=wo[:, :, :], in_=dWo[:, :, :])
            nc.sync.dma_start(out=bq, in_=dBQ[:, :])
            nc.sync.dma_start(out=bk, in_=dBK[:, :])
            nc.sync.dma_start(out=bo2, in_=dBO2[:, :])
            nc.sync.dma_start(out=bv, in_=dBV[:, :])

            # ones column for the fused softmax denominator
            nc.vector.memset(vpa[:, :, :, DH:DH + 1], 1.0)

            def project(dst, w, rhs_src, bias_ap, t, n):
                """dst[:, t, nsl] = (w[:,:,t-tile].T @ rhs_src[:,:,nsl]) + bias, bf16."""
                nsl = slice(n * 512, (n + 1) * 512)
                pp = ppool.tile([128, 512], F32, name="pp", tag="pp")
                for kc in range(KC):
                    nc.tensor.matmul(
                        pp[:, :],
                        w[:, kc, t * 128:(t + 1) * 128],
                        rhs_src[:, kc, nsl],
                        start=(kc == 0),
                        stop=(kc == KC - 1),
                    )
                nc.vector.tensor_scalar(dst[:, t, nsl], pp[:, :], bias_ap, None, ALU.add)

            def project_v(mt):
                """vpa[:, mt, :, 0:64] = (kt[:,:,mt-tile].T @ wv) in bf16."""
                pv = ppool.tile([128, 512], F32, name="pv", tag="pp")
                for kc in range(KC):
                    nc.tensor.matmul(
                        pv[:, :],
                        kt[:, kc, mt * 128:(mt + 1) * 128],
                        wv[:, kc, :],
                        start=(kc == 0),
                        stop=(kc == KC - 1),
                    )
                nc.vector.tensor_copy(
                    vpa[:, mt, :, 0:DH],
                    pv[:, :].rearrange("p (h d) -> p h d", h=H),
                )

            def ffn(n, mo):
                """dOT[mo-tile, nsl] = o1 + bv + relu(wo.T @ o1 + bo2)."""
                nsl = slice(n * 512, (n + 1) * 512)
                pf = ppool.tile([128, 512], F32, name="pf", tag="pp")
                for kc in range(KC):
                    nc.tensor.matmul(
                        pf[:, :],
                        wo[:, kc, mo * 128:(mo + 1) * 128],
                        o1[:, kc, nsl],
                        start=(kc == 0),
                        stop=(kc == KC - 1),
                    )
                rf = fpool.tile([128, 512], F32, name="rf", tag="rf")
                nc.vector.tensor_scalar(
                    rf[:, :], pf[:, :], bo2[:, mo:mo + 1], 0.0, ALU.add, ALU.max
                )
                ot = outpool.tile([128, 512], F32, name="ot", tag="ot")
                nc.vector.scalar_tensor_tensor(
                    ot[:, :], rf[:, :], bv[:, mo:mo + 1], o1[:, mo, nsl],
                    ALU.add, ALU.add,
                )
                h0 = slice(n * 512, n * 512 + 256)
                h1 = slice(n * 512 + 256, (n + 1) * 512)
                e1 = nc.scalar if n == 1 else nc.sync
                nc.gpsimd.dma_start(out=dOT[mo * 128:(mo + 1) * 128, h0], in_=ot[:, 0:256])
                e1.dma_start(out=dOT[mo * 128:(mo + 1) * 128, h1], in_=ot[:, 256:512])

            # deferred work units, drip-fed into attention PE gaps
            fillers = []

            def attend_pair(n, t, fill_sched):
                """Heads 2t (scores rows 0-63) and 2t+1 (rows 64-127), q-chunk n.

                fill_sched: pops-per-iteration list (len KT8+1); fillers a pair's
                scores depend on must be popped in an EARLIER pair (in-order
                engine programs), except project_v(m) which may pop at iter <= m
                of pair (0,0) since ctx(m) is emitted at iter m+1.
                """
                nsl = slice(n * 512, (n + 1) * 512)
                pc = cpool.tile([DH + 1, 2, 512], F32, name="pc", tag="pc")
                # ctx window: accumulate ctx in contiguous runs of W k-tiles per
                # head so the A/B PSUM accumulation groups never interleave
                # (interleaved groups corrupt PSUM on HW; cf. v1 docstring)
                W = int(os.environ.get("KCTXWIN", "0"))
                ps_q = []   # software pipeline: scores ahead of ctx
                for m in range(KT8 + 1):
                    npop = fill_sched[m] if m < len(fill_sched) else 0
                    if m < KT8:
                        ps = spool.tile([128, 2, 512], F32, name="ps", tag="ps")
                        nc.tensor.matmul(
                            ps[:, 0, :],
                            kpb[0:64, t, m * 128:(m + 1) * 128],
                            qp[0:64, t, nsl],
                            start=True, stop=True,
                        )
                        nc.tensor.matmul(
                            ps[:, 1, :],
                            kpb[64:128, t, m * 128:(m + 1) * 128],
                            qp[64:128, t, nsl],
                            start=True, stop=True,
                        )
                        ex = epool.tile([128, 2, 512], BF16, name="ex", tag="ex")
                        nc.scalar.activation(
                            ex[:, :, :], ps[:, :, :], ACTF.Exp, scale=scale
                        )
                        if dbg and n == 0 and t == 0 and m == 0:
                            nc.sync.dma_start(out=dDEX[:, :, :], in_=ex[:, :, :])
                        ps_q.append((m, ex))
                    if W == 0:
                        if m >= 1:
                            mm, ex = ps_q.pop(0)
                            for hh in range(2):
                                nc.tensor.matmul(
                                    pc[:, hh, :],
                                    vpa[:, mm, 2 * t + hh, :],
                                    ex[:, hh, :],
                                    start=(mm == 0),
                                    stop=(mm == KT8 - 1),
                                )
                    elif m < KT8 and (m + 1) % W == 0:
                        batch, ps_q = ps_q, []
                        for hh in range(2):
                            for mm, ex in batch:
                                nc.tensor.matmul(
                                    pc[:, hh, :],
                                    vpa[:, mm, 2 * t + hh, :],
                                    ex[:, hh, :],
                                    start=(mm == 0),
                                    stop=(mm == KT8 - 1),
                                )
                    for _ in range(npop):
                        if fillers:
                            fillers.pop(0)()

                normalize(n, t, pc)

            def normalize(n, t, pc):
                nsl = slice(n * 512, (n + 1) * 512)
                # ---- normalize + residual for this (n, t) ----
                # Z rows 0-63: ctx_A, row 64: denom_A (later rows 64-127 <- ctx_B)
                Z = zpool.tile([128, 512], F32, name="Z", tag="Z")
                Y = ypool.tile([DH + 1, 512], F32, name="Y", tag="Y")
                nc.vector.tensor_copy(Z[0:DH + 1, :], pc[:, 0, :])
                nc.vector.tensor_copy(Y[:, :], pc[:, 1, :])
                # whole-tile reciprocals (single-partition DVE slices at a
                # nonzero base partition misexecute at partition 0 -- only the
                # row-64 result is used)
                R = rpool.tile([DH + 1, 1024], F32, name="R", tag="R")
                nc.vector.reciprocal_approx_fast(R[:, 0:512], Z[0:DH + 1, :])
                nc.vector.reciprocal_approx_fast(R[:, 512:1024], Y[:, :])
                # DRAM-bounce the recip denom row, broadcast back to 64
                # partitions each (partition-step-0 APs are DRAM-only)
                rec_d = dpool.tile([1, 1024], F32, name="rec_d", tag="rec_d")
                nc.sync.dma_start(out=rec_d[:, :], in_=R[DH:DH + 1, :])
                rb = rbpool.tile([128, 512], F32, name="rb", tag="rb")
                srcA = bass.AP(
                    tensor=rec_d[0:1, 0:512].tensor,
                    offset=rec_d[0:1, 0:512].offset,
                    ap=[[0, 64], [1, 512]],
                )
                srcB = bass.AP(
                    tensor=rec_d[0:1, 512:1024].tensor,
                    offset=rec_d[0:1, 512:1024].offset,
                    ap=[[0, 64], [1, 512]],
                )
                nc.sync.dma_start(out=rb[0:64, :], in_=srcA)
                nc.sync.dma_start(out=rb[64:128, :], in_=srcB)
                # shift ctx_B into partitions 64-127 (overwrites denom_A row too)
                nc.gpsimd.dma_start(out=Z[64:128, :], in_=Y[0:DH, :])
                cn = cnpool.tile([128, 512], BF16, name="cn", tag="cn")
                nc.vector.tensor_mul(cn[:, :], Z[:, :], rb[:, :])
                nc.vector.tensor_add(o1[:, t, nsl], cn[:, :], qp[:, t, nsl])
                if dbg and n == 0 and t == 0:
                    nc.sync.dma_start(out=dDCU[:, :], in_=Z[:, :])
                    nc.sync.dma_start(out=dDRB[:, :], in_=rb[:, :])

            # ---- emission ----
            # minimal upfront: pair (0,0) prerequisites (kpb keys 512+ are
            # needed only from k-tile m=4 -> kp(t0,k1) is filler iter 0)
            project(qp, wq, qt, bq[:, 0:1], 0, 0)
            project(kpb, wk, kt, bk[:, 0:1], 0, 0)

            if os.environ.get("KNOFILL", "0") == "1":
                # bisect mode: no filler interleaving; everything upfront
                for t in range(MT):
                    for nn in range(NQ):
                        if (t, nn) != (0, 0):
                            project(qp, wq, qt, bq[:, t:t + 1], t, nn)
                            project(kpb, wk, kt, bk[:, t:t + 1], t, nn)
                for mt in range(KT8):
                    project_v(mt)
                for n in range(NQ):
                    for t in range(MT):
                        attend_pair(n, t, [0] * (KT8 + 1))
                for n in range(NQ):
                    for mo in range(MT):
                        ffn(n, mo)
                if dbg:
                    nc.sync.dma_start(out=dDQP[:, :, :], in_=qp[:, :, :])
                    nc.sync.dma_start(out=dDKP[:, :, :], in_=kpb[:, :, :])
                    nc.sync.dma_start(out=dDO1[:, :, :], in_=o1[:, :, :])
                emit_main = False
            else:
                emit_main = True

            # fillers, in dependency order. Pair (0,0) interleaves the t=1/n=0
            # projections between the project_v units (pv(m) must pop by iter m
            # of pair (0,0); the t1 projections by end of pair (0,0)); every
            # other unit is popped at least one pair before its consumer.
            if emit_main:
                # kpb is indexed by KEY chunk (both halves needed by iter 4 of
                # ANY pair of that t); qp by q-chunk n. pv(m) must pop by iter
                # m of pair (0,0); all else pops >=1 pair before its consumer.
                fillers.append(lambda: project(kpb, wk, kt, bk[:, 0:1], 0, 1))
                fillers.append(lambda: project_v(0))
                fillers.append(lambda: project(kpb, wk, kt, bk[:, 1:2], 1, 0))
                fillers.append(lambda: project_v(1))
                fillers.append(lambda: project(kpb, wk, kt, bk[:, 1:2], 1, 1))
                fillers.append(lambda: project_v(2))
                fillers.append(lambda: project(qp, wq, qt, bq[:, 1:2], 1, 0))
                for mt in range(3, KT8):
                    fillers.append(lambda mt=mt: project_v(mt))
                for t in range(2, MT):
                    fillers.append(lambda t=t: project(kpb, wk, kt, bk[:, t:t + 1], t, 0))
                    fillers.append(lambda t=t: project(kpb, wk, kt, bk[:, t:t + 1], t, 1))
                    fillers.append(lambda t=t: project(qp, wq, qt, bq[:, t:t + 1], t, 0))
                for t in range(MT):
                    fillers.append(lambda t=t: project(qp, wq, qt, bq[:, t:t + 1], t, 1))

                scheds = {
                    (0, 0): [2, 2, 2, 2, 1, 1, 1, 1, 0],  # kp(t0,k1), 8 pv, t1 set
                    (0, 1): [1, 1, 1, 0, 0, 0, 0, 0, 0],  # t2 set
                    (0, 2): [1, 1, 1, 0, 0, 0, 0, 0, 0],  # t3 set
                    (0, 3): [1, 0, 0, 0, 0, 0, 0, 0, 0],  # qp(t0, n1)
                    (1, 0): [1, 0, 0, 0, 0, 0, 0, 0, 0],  # qp(t1, n1)
                    (1, 1): [1, 1, 0, 0, 0, 0, 0, 0, 0],  # qp(t2/t3, n1)
                    (1, 2): [1, 1, 1, 0, 0, 0, 0, 0, 0],  # ffn(0, 0/1/2)
                    (1, 3): [1, 0, 0, 0, 0, 0, 0, 0, 0],  # ffn(0, 3)
                }
                # flat pipeline over all (n, t, m): each slot emits the NEXT
                # unit's scores+exp BEFORE the previous unit's ctx, so a new
                # pair's scores sit ahead of the old pair's last ctx in the PE
                # stream and the ACT never waits out a pair boundary.
                pairs = [(n, t) for n in range(NQ) for t in range(MT)]
                flat_pops = []
                for p in pairs:
                    flat_pops.extend(scheds[p][:KT8])
                flat_pops.append(0)
                units = [(n, t, m) for (n, t) in pairs for m in range(KT8)]
                exq = []
                pc_cur = [None]
                for sl in range(len(units) + 1):
                    if sl < len(units):
                        n, t, m = units[sl]
                        nsl = slice(n * 512, (n + 1) * 512)
                        ps = spool.tile([128, 2, 512], F32, name="ps", tag="ps")
                        nc.tensor.matmul(
                            ps[:, 0, :],
                            kpb[0:64, t, m * 128:(m + 1) * 128],
                            qp[0:64, t, nsl], start=True, stop=True,
                        )
                        nc.tensor.matmul(
                            ps[:, 1, :],
                            kpb[64:128, t, m * 128:(m + 1) * 128],
                            qp[64:128, t, nsl], start=True, stop=True,
                        )
                        ex = epool.tile([128, 2, 512], BF16, name="ex", tag="ex")
                        nc.scalar.activation(
                            ex[:, :, :], ps[:, :, :], ACTF.Exp, scale=scale
                        )
                        if dbg and sl == 0:
                            nc.sync.dma_start(out=dDEX[:, :, :], in_=ex[:, :, :])
                        exq.append(ex)
                    if sl >= 1:
                        n0, t0, m0 = units[sl - 1]
                        ex = exq.pop(0)
                        if m0 == 0:
                            pc_cur[0] = cpool.tile(
                                [DH + 1, 2, 512], F32, name="pc", tag="pc"
                            )
                        pc = pc_cur[0]
                        for hh in range(2):
                            nc.tensor.matmul(
                                pc[:, hh, :],
                                vpa[:, m0, 2 * t0 + hh, :],
                                ex[:, hh, :],
                                start=(m0 == 0), stop=(m0 == KT8 - 1),
                            )
                        if m0 == KT8 - 1:
                            normalize(n0, t0, pc)
                            if (n0, t0) == (0, MT - 1):
                                # FFN for n=0 becomes filler work during n=1
                                for mo in range(MT):
                                    fillers.append(lambda mo=mo: ffn(0, mo))
                    for _ in range(flat_pops[sl]):
                        if fillers:
                            fillers.pop(0)()
                while fillers:
                    fillers.pop(0)()
                for mo in range(MT):
                    ffn(1, mo)

                if dbg:
                    nc.sync.dma_start(out=dDQP[:, :, :], in_=qp[:, :, :])
                    nc.sync.dma_start(out=dDKP[:, :, :], in_=kpb[:, :, :])
                    nc.sync.dma_start(out=dDO1[:, :, :], in_=o1[:, :, :])

    nc.compile()
    return nc


def _get_nc():
    global _NC
    if _NC is None:
        _NC = _build()
    return _NC


def _prep_inputs(Q, K, Wq, bq, Wk, bk, Wv, bv, Wo, bo):
    import ml_dtypes
    bf = ml_dtypes.bfloat16

    Q = np.asarray(Q, dtype=np.float32)
    K = np.asarray(K, dtype=np.float32)
    Wq = np.asarray(Wq, dtype=np.float32)
    Wk = np.asarray(Wk, dtype=np.float32)
    Wv = np.asarray(Wv, dtype=np.float32)
    Wo = np.asarray(Wo, dtype=np.float32)
    bq = np.asarray(bq, dtype=np.float32)
    bk = np.asarray(bk, dtype=np.float32)
    bv = np.asarray(bv, dtype=np.float32)
    bo = np.asarray(bo, dtype=np.float32)

    bo2 = (bo + Wo @ bv).astype(np.float32)

    def btile(b):
        return np.ascontiguousarray(b.reshape(MT, 128).T)

    def pack(wT, free):
        # [din, free] -> SBUF layout [128, KC, free]
        return np.ascontiguousarray(
            wT.reshape(KC, 128, free).transpose(1, 0, 2)
        ).astype(bf)

    shared = {
        "WqT": pack(Wq.T, D),
        "WkT": pack(Wk.T, D),
        "WvT": pack(Wv.T, D),
        "WoT": pack(Wo.T, D),
        "BQ": btile(bq),
        "BK": btile(bk),
        "BO2": btile(bo2),
        "BV": btile(bv),
    }
    in_maps = []
    for c in range(N_CORES):
        m = dict(shared)
        m["QT"] = pack(Q[c].T, SQ)
        m["KT"] = pack(K[c].T, SK)
        in_maps.append(m)
    return in_maps


def run(inputs, trace=False):
    """Run on hardware; returns (output [B,SQ,D] f32, BassKernelResults)."""
    in_maps = _prep_inputs(
        inputs["Q"], inputs["K"], inputs["Wq"], inputs["bq"], inputs["Wk"],
        inputs["bk"], inputs["Wv"], inputs["bv"], inputs["Wo"], inputs["bo"],
    )
    nc = _get_nc()
    res = run_bass_kernel_spmd(
        nc, in_maps, core_ids=list(range(N_CORES)), trace=trace
    )
    out = np.stack(
        [res.results[c]["OT"].T for c in range(N_CORES)], axis=0
    ).astype(np.float32)
    return out, res


def kernel(**inputs):
    nh = inputs.get("num_heads", H)
    assert int(nh) == H, f"kernel hardcodes num_heads={H}, got {nh}"
    out, _ = run(inputs, trace=False)
    return out


if __name__ == "__main__":
    rng = np.random.default_rng(0)
    inputs = {
        "Q": rng.standard_normal((B, SQ, D), dtype=np.float32),
        "K": rng.standard_normal((B, SK, D), dtype=np.float32),
        "Wq": rng.standard_normal((D, D), dtype=np.float32) * 0.04,
        "bq": rng.standard_normal((D,), dtype=np.float32) * 0.04,
        "Wk": rng.standard_normal((D, D), dtype=np.float32) * 0.04,
        "bk": rng.standard_normal((D,), dtype=np.float32) * 0.04,
        "Wv": rng.standard_normal((D, D), dtype=np.float32) * 0.04,
        "bv": rng.standard_normal((D,), dtype=np.float32) * 0.04,
        "Wo": rng.standard_normal((D, D), dtype=np.float32) * 0.04,
        "bo": rng.standard_normal((D,), dtype=np.float32) * 0.04,
        "num_heads": H,
    }
    out = kernel(**inputs)
    print("out", out.shape, out.dtype, float(np.abs(out).max()))


# revision 5
# speedup vs baseline: 1.0602x; 1.0602x over previous
"""Trainium2 Bass kernel for nn_MAB_44057774522768 (Set-Transformer MAB block), v2.

Per batch b (B=8 -> core b), Sq=Sk=1024, D=512, H=8 heads, dh=64:
    Qp = Q @ Wq.T + bq;  Kp = K @ Wk.T + bk;  Vp = K @ Wv.T (bv folded out)
    scores_h = Qp_h @ Kp_h.T / sqrt(D);  A = softmax(scores)
    ctx_h = A_h @ Vp_h;  O1 = Qp + ctx
    out = O1 + bv + relu(O1 @ Wo.T + bo + Wo@bv)

v2 design (vs v1 baseline at ~180us/core):
  * all-bf16 inputs/weights (halves HBM traffic; f32 only in PSUM + final out)
  * scores for a head pair run CONCURRENTLY as K=64 row-tiles (partitions
    0-63 / 64-127) -> 2x on the score matmuls
  * scores output bf16 directly to PSUM (1KB/bank), exp'd in one ACT op per
    (pair, k-tile) of FD=1024 -> ACT (the 73us exp floor) stays saturated
  * q-chunked dataflow (n = 512-col chunk outer loop): ctx accumulators are
    [65, 512] -> whole pipeline fits in 8 PSUM banks with double-buffering,
    and FFN(n=0) + its output DMA overlap n=1's attention
  * softmax denominator via ones-column in Vp (row 64 of ctx PSUM);
    whole-tile fp32 reciprocal (single-partition DVE slices at base partition
    64 silently execute at partition 0 -- never slice-recip row 64 alone),
    then DRAM-bounce + partition-step-0 broadcast DMA back to 64 partitions
  * projections and FFN drip-fed into attention's PE gaps as fillers
"""

import math
import os

import numpy as np

import concourse.bass as bass
import concourse.mybir as mybir
import concourse.tile as tile
from concourse import bacc
from concourse.bass_utils import run_bass_kernel_spmd

B, SQ, SK, D = 8, 1024, 1024, 512
H, DH = 8, 64
N_CORES = 8
KC = D // 128   # 4 contraction chunks of 128 (din)
MT = D // 128   # 4 output-feature tiles of 128 (dout)
NQ = SQ // 512  # 2 q chunks of 512
KT8 = SK // 128 # 8 key-seq tiles of 128

F32 = mybir.dt.float32
BF16 = mybir.dt.bfloat16
ALU = mybir.AluOpType
ACTF = mybir.ActivationFunctionType

_NC = None


def _build():
    nc = bacc.Bacc(None, target_bir_lowering=False, debug=False)

    # inputs host-packed to the exact SBUF layout [partition, kc, free] so
    # each loads in ONE descriptor with 4-8KB partition lines (the 3 HWDGE
    # queues run ~20GB/s on 1KB lines but much faster on big contiguous ones)
    dQT = nc.dram_tensor("QT", [128, 2, KC, SQ // 2], BF16, kind="ExternalInput")
    dKT = nc.dram_tensor("KT", [128, 2, KC, SK // 2], BF16, kind="ExternalInput")
    dWq = nc.dram_tensor("WqT", [128, KC, D], BF16, kind="ExternalInput")
    dWk = nc.dram_tensor("WkT", [128, KC, D], BF16, kind="ExternalInput")
    dWv = nc.dram_tensor("WvT", [128, KC, D], BF16, kind="ExternalInput")
    dWo = nc.dram_tensor("WoT", [128, KC, D], BF16, kind="ExternalInput")
    dBQ = nc.dram_tensor("BQ", [128, MT], F32, kind="ExternalInput")
    dBK = nc.dram_tensor("BK", [128, MT], F32, kind="ExternalInput")
    dBO2 = nc.dram_tensor("BO2", [128, MT], F32, kind="ExternalInput")
    dBV = nc.dram_tensor("BV", [128, MT], F32, kind="ExternalInput")
    dOT = nc.dram_tensor("OT", [D, SQ], F32, kind="ExternalOutput")

    dbg = os.environ.get("KDEBUG", "0") == "1"
    if dbg:
        dDQP = nc.dram_tensor("DQP", [128, MT, SQ], BF16, kind="ExternalOutput")
        dDKP = nc.dram_tensor("DKP", [128, MT, SK], BF16, kind="ExternalOutput")
        dDEX = nc.dram_tensor("DEX", [128, 2, 512], BF16, kind="ExternalOutput")
        dDO1 = nc.dram_tensor("DO1", [128, MT, SQ], BF16, kind="ExternalOutput")
        dDCU = nc.dram_tensor("DCU", [128, 512], F32, kind="ExternalOutput")
        dDRB = nc.dram_tensor("DRB", [128, 512], F32, kind="ExternalOutput")

    scale = 1.0 / math.sqrt(float(D))
    ctx_win = int(os.environ.get("KCTXWIN", "0"))

    with tile.TileContext(nc) as tc:
        with (
            tc.tile_pool(name="persist", bufs=1) as persist,
            tc.tile_pool(name="ppool", bufs=2, space="PSUM") as ppool,   # 1 bank x2
            tc.tile_pool(name="spool", bufs=2, space="PSUM") as spool,   # 2 banks x2
            tc.tile_pool(name="cpool", bufs=1, space="PSUM") as cpool,   # 2 banks
            tc.tile_pool(name="epool", bufs=max(3, ctx_win + 2)) as epool,
            tc.tile_pool(name="zpool", bufs=2) as zpool,
            tc.tile_pool(name="ypool", bufs=2) as ypool,
            tc.tile_pool(name="rpool", bufs=2) as rpool,
            tc.tile_pool(name="rbpool", bufs=2) as rbpool,
            tc.tile_pool(name="dpool", bufs=2, space="DRAM") as dpool,
            tc.tile_pool(name="cnpool", bufs=2) as cnpool,
            tc.tile_pool(name="fpool", bufs=2) as fpool,
            tc.tile_pool(name="outpool", bufs=3) as outpool,
        ):
            # ---- persistent SBUF tensors ----
            qt = persist.tile([128, 2, KC, SQ // 2], BF16)  # half-major
            kt = persist.tile([128, 2, KC, SK // 2], BF16)
            wq = persist.tile([128, KC, D], BF16)
            wk = persist.tile([128, KC, D], BF16)
            wv = persist.tile([128, KC, D], BF16)
            wo = persist.tile([128, KC, D], BF16)
            bq = persist.tile([128, MT], F32)
            bk = persist.tile([128, MT], F32)
            bo2 = persist.tile([128, MT], F32)
            bv = persist.tile([128, MT], F32)
            qp = persist.tile([128, MT, SQ], BF16)   # projected Q (scores rhs + residual)
            kpb = persist.tile([128, MT, SK], BF16)
            # Vp seq-major [k, kt8, h, dh] + ones column at dh=64 per head
            vpa = persist.tile([128, KT8, H, DH + 1], BF16)
            o1 = persist.tile([128, MT, SQ], BF16)

            # ---- input DMAs: one big descriptor per tensor, spread across
            # the 3 queues by first-need (wq/wk gate the upfront projections,
            # qt/kt gate scores, wv gates ctx, wo is needed only by the FFN)
            # sync (SP) is the slow software queue (~50GB/s) -- biases only.
            # scalar + gpsimd are fast hardware queues (~150-250GB/s): weights
            # first (they gate the upfront projections), then activations.
            nc.scalar.dma_start(out=wq[:, :, :], in_=dWq[:, :, :])
            nc.gpsimd.dma_start(out=wk[:, :, :], in_=dWk[:, :, :])
            nc.scalar.dma_start(out=qt[:, 0, :, :], in_=dQT[:, 0, :, :])
            nc.gpsimd.dma_start(out=kt[:, 0, :, :], in_=dKT[:, 0, :, :])
            nc.scalar.dma_start(out=wv[:, :, :], in_=dWv[:, :, :])
            nc.gpsimd.dma_start(out=kt[:, 1, :, :], in_=dKT[:, 1, :, :])
            nc.scalar.dma_start(out=qt[:, 1, :, :], in_=dQT[:, 1, :, :])
            nc.gpsimd.dma_start(out=wo[:, :, :], in_=dWo[:, :, :])
            nc.sync.dma_start(out=bq, in_=dBQ[:, :])
            nc.sync.dma_start(out=bk, in_=dBK[:, :])
            nc.sync.dma_start(out=bo2, in_=dBO2[:, :])
            nc.sync.dma_start(out=bv, in_=dBV[:, :])

            # ones column for the fused softmax denominator
            nc.vector.memset(vpa[:, :, :, DH:DH + 1], 1.0)

            def project(dst, w, rhs_src, bias_ap, t, n):
                """dst[:, t, nsl] = (w[:,:,t-tile].T @ rhs_src[:,:,nsl]) + bias, bf16."""
                nsl = slice(n * 512, (n + 1) * 512)
                pp = ppool.tile([128, 512], F32, name="pp", tag="pp")
                for kc in range(KC):
                    nc.tensor.matmul(
                        pp[:, :],
                        w[:, kc, t * 128:(t + 1) * 128],
                        rhs_src[:, n, kc, :],
                        start=(kc == 0),
                        stop=(kc == KC - 1),
                    )
                nc.vector.tensor_scalar(dst[:, t, nsl], pp[:, :], bias_ap, None, ALU.add)

            def project_v(mt):
                """vpa[:, mt, :, 0:64] = (kt[:,:,mt-tile].T @ wv) in bf16."""
                pv = ppool.tile([128, 512], F32, name="pv", tag="pp")
                mh, mo_ = mt // 4, mt % 4
                for kc in range(KC):
                    nc.tensor.matmul(
                        pv[:, :],
                        kt[:, mh, kc, mo_ * 128:(mo_ + 1) * 128],
                        wv[:, kc, :],
                        start=(kc == 0),
                        stop=(kc == KC - 1),
                    )
                nc.vector.tensor_copy(
                    vpa[:, mt, :, 0:DH],
                    pv[:, :].rearrange("p (h d) -> p h d", h=H),
                )

            def ffn(n, mo):
                """dOT[mo-tile, nsl] = o1 + bv + relu(wo.T @ o1 + bo2)."""
                nsl = slice(n * 512, (n + 1) * 512)
                pf = ppool.tile([128, 512], F32, name="pf", tag="pp")
                for kc in range(KC):
                    nc.tensor.matmul(
                        pf[:, :],
                        wo[:, kc, mo * 128:(mo + 1) * 128],
                        o1[:, kc, nsl],
                        start=(kc == 0),
                        stop=(kc == KC - 1),
                    )
                rf = fpool.tile([128, 512], F32, name="rf", tag="rf")
                nc.vector.tensor_scalar(
                    rf[:, :], pf[:, :], bo2[:, mo:mo + 1], 0.0, ALU.add, ALU.max
                )
                ot = outpool.tile([128, 512], F32, name="ot", tag="ot")
                nc.vector.scalar_tensor_tensor(
                    ot[:, :], rf[:, :], bv[:, mo:mo + 1], o1[:, mo, nsl],
                    ALU.add, ALU.add,
                )
                rowsl = slice(mo * 128, (mo + 1) * 128)
                if n == 0:
                    nc.gpsimd.dma_start(out=dOT[rowsl, 0:256], in_=ot[:, 0:256])
                    nc.sync.dma_start(out=dOT[rowsl, 256:512], in_=ot[:, 256:512])
                else:
                    nc.gpsimd.dma_start(out=dOT[rowsl, 512:768], in_=ot[:, 0:256])
                    nc.scalar.dma_start(out=dOT[rowsl, 768:896], in_=ot[:, 256:384])
                    nc.sync.dma_start(out=dOT[rowsl, 896:1024], in_=ot[:, 384:512])

            # deferred work units, drip-fed into attention PE gaps
            fillers = []

            def attend_pair(n, t, fill_sched):
                """Heads 2t (scores rows 0-63) and 2t+1 (rows 64-127), q-chunk n.

                fill_sched: pops-per-iteration list (len KT8+1); fillers a pair's
                scores depend on must be popped in an EARLIER pair (in-order
                engine programs), except project_v(m) which may pop at iter <= m
                of pair (0,0) since ctx(m) is emitted at iter m+1.
                """
                nsl = slice(n * 512, (n + 1) * 512)
                pc = cpool.tile([DH + 1, 2, 512], F32, name="pc", tag="pc")
                # ctx window: accumulate ctx in contiguous runs of W k-tiles per
                # head so the A/B PSUM accumulation groups never interleave
                # (interleaved groups corrupt PSUM on HW; cf. v1 docstring)
                W = int(os.environ.get("KCTXWIN", "0"))
                ps_q = []   # software pipeline: scores ahead of ctx
                for m in range(KT8 + 1):
                    npop = fill_sched[m] if m < len(fill_sched) else 0
                    if m < KT8:
                        ps = spool.tile([128, 2, 512], F32, name="ps", tag="ps")
                        nc.tensor.matmul(
                            ps[:, 0, :],
                            kpb[0:64, t, m * 128:(m + 1) * 128],
                            qp[0:64, t, nsl],
                            start=True, stop=True,
                        )
                        nc.tensor.matmul(
                            ps[:, 1, :],
                            kpb[64:128, t, m * 128:(m + 1) * 128],
                            qp[64:128, t, nsl],
                            start=True, stop=True,
                        )
                        ex = epool.tile([128, 2, 512], BF16, name="ex", tag="ex")
                        nc.scalar.activation(
                            ex[:, :, :], ps[:, :, :], ACTF.Exp, scale=scale
                        )
                        if dbg and n == 0 and t == 0 and m == 0:
                            nc.sync.dma_start(out=dDEX[:, :, :], in_=ex[:, :, :])
                        ps_q.append((m, ex))
                    if W == 0:
                        if m >= 1:
                            mm, ex = ps_q.pop(0)
                            for hh in range(2):
                                nc.tensor.matmul(
                                    pc[:, hh, :],
                                    vpa[:, mm, 2 * t + hh, :],
                                    ex[:, hh, :],
                                    start=(mm == 0),
                                    stop=(mm == KT8 - 1),
                                )
                    elif m < KT8 and (m + 1) % W == 0:
                        batch, ps_q = ps_q, []
                        for hh in range(2):
                            for mm, ex in batch:
                                nc.tensor.matmul(
                                    pc[:, hh, :],
                                    vpa[:, mm, 2 * t + hh, :],
                                    ex[:, hh, :],
                                    start=(mm == 0),
                                    stop=(mm == KT8 - 1),
                                )
                    for _ in range(npop):
                        if fillers:
                            fillers.pop(0)()

                normalize(n, t, pc)

            def normalize(n, t, pc):
                nsl = slice(n * 512, (n + 1) * 512)
                # ---- normalize + residual for this (n, t) ----
                # Z rows 0-63: ctx_A, row 64: denom_A (later rows 64-127 <- ctx_B)
                Z = zpool.tile([128, 512], F32, name="Z", tag="Z")
                Y = ypool.tile([DH + 1, 512], F32, name="Y", tag="Y")
                nc.vector.tensor_copy(Z[0:DH + 1, :], pc[:, 0, :])
                nc.vector.tensor_copy(Y[:, :], pc[:, 1, :])
                # whole-tile reciprocals (single-partition DVE slices at a
                # nonzero base partition misexecute at partition 0 -- only the
                # row-64 result is used)
                R = rpool.tile([DH + 1, 1024], F32, name="R", tag="R")
                nc.vector.reciprocal_approx_fast(R[:, 0:512], Z[0:DH + 1, :])
                nc.vector.reciprocal_approx_fast(R[:, 512:1024], Y[:, :])
                # DRAM-bounce the recip denom row, broadcast back to 64
                # partitions each (partition-step-0 APs are DRAM-only)
                rec_d = dpool.tile([1, 1024], F32, name="rec_d", tag="rec_d")
                nc.sync.dma_start(out=rec_d[:, :], in_=R[DH:DH + 1, :])
                rb = rbpool.tile([128, 512], F32, name="rb", tag="rb")
                srcA = bass.AP(
                    tensor=rec_d[0:1, 0:512].tensor,
                    offset=rec_d[0:1, 0:512].offset,
                    ap=[[0, 64], [1, 512]],
                )
                srcB = bass.AP(
                    tensor=rec_d[0:1, 512:1024].tensor,
                    offset=rec_d[0:1, 512:1024].offset,
                    ap=[[0, 64], [1, 512]],
                )
                nc.sync.dma_start(out=rb[0:64, :], in_=srcA)
                nc.sync.dma_start(out=rb[64:128, :], in_=srcB)
                # shift ctx_B into partitions 64-127 (overwrites denom_A row too)
                nc.gpsimd.dma_start(out=Z[64:128, :], in_=Y[0:DH, :])
                cn = cnpool.tile([128, 512], BF16, name="cn", tag="cn")
                nc.vector.tensor_mul(cn[:, :], Z[:, :], rb[:, :])
                nc.vector.tensor_add(o1[:, t, nsl], cn[:, :], qp[:, t, nsl])
                if dbg and n == 0 and t == 0:
                    nc.sync.dma_start(out=dDCU[:, :], in_=Z[:, :])
                    nc.sync.dma_start(out=dDRB[:, :], in_=rb[:, :])

            # ---- emission ----
            # minimal upfront: pair (0,0) prerequisites (kpb keys 512+ are
            # needed only from k-tile m=4 -> kp(t0,k1) is filler iter 0)
            project(qp, wq, qt, bq[:, 0:1], 0, 0)
            project(kpb, wk, kt, bk[:, 0:1], 0, 0)

            if os.environ.get("KNOFILL", "0") == "1":
                # bisect mode: no filler interleaving; everything upfront
                for t in range(MT):
                    for nn in range(NQ):
                        if (t, nn) != (0, 0):
                            project(qp, wq, qt, bq[:, t:t + 1], t, nn)
                            project(kpb, wk, kt, bk[:, t:t + 1], t, nn)
                for mt in range(KT8):
                    project_v(mt)
                for n in range(NQ):
                    for t in range(MT):
                        attend_pair(n, t, [0] * (KT8 + 1))
                for n in range(NQ):
                    for mo in range(MT):
                        ffn(n, mo)
                if dbg:
                    nc.sync.dma_start(out=dDQP[:, :, :], in_=qp[:, :, :])
                    nc.sync.dma_start(out=dDKP[:, :, :], in_=kpb[:, :, :])
                    nc.sync.dma_start(out=dDO1[:, :, :], in_=o1[:, :, :])
                emit_main = False
            else:
                emit_main = True

            # fillers, in dependency order. Pair (0,0) interleaves the t=1/n=0
            # projections between the project_v units (pv(m) must pop by iter m
            # of pair (0,0); the t1 projections by end of pair (0,0)); every
            # other unit is popped at least one pair before its consumer.
            if emit_main:
                # kpb is indexed by KEY chunk (both halves needed by iter 4 of
                # ANY pair of that t); qp by q-chunk n. pv(m) must pop by iter
                # m of pair (0,0); all else pops >=1 pair before its consumer.
                fillers.append(lambda: project(kpb, wk, kt, bk[:, 0:1], 0, 1))
                fillers.append(lambda: project_v(0))
                fillers.append(lambda: project(kpb, wk, kt, bk[:, 1:2], 1, 0))
                fillers.append(lambda: project_v(1))
                fillers.append(lambda: project(kpb, wk, kt, bk[:, 1:2], 1, 1))
                fillers.append(lambda: project_v(2))
                fillers.append(lambda: project(qp, wq, qt, bq[:, 1:2], 1, 0))
                for mt in range(3, KT8):
                    fillers.append(lambda mt=mt: project_v(mt))
                for t in range(2, MT):
                    fillers.append(lambda t=t: project(kpb, wk, kt, bk[:, t:t + 1], t, 0))
                    fillers.append(lambda t=t: project(kpb, wk, kt, bk[:, t:t + 1], t, 1))
                    fillers.append(lambda t=t: project(qp, wq, qt, bq[:, t:t + 1], t, 0))
                for t in range(MT):
                    fillers.append(lambda t=t: project(qp, wq, qt, bq[:, t:t + 1], t, 1))

                scheds = {
                    (0, 0): [2, 2, 2, 2, 1, 1, 1, 1, 0],  # kp(t0,k1), 8 pv, t1 set
                    (0, 1): [1, 1, 1, 0, 0, 0, 0, 0, 0],  # t2 set
                    (0, 2): [1, 1, 1, 0, 0, 0, 0, 0, 0],  # t3 set
                    (0, 3): [1, 0, 0, 0, 0, 0, 0, 0, 0],  # qp(t0, n1)
                    (1, 0): [1, 0, 0, 0, 0, 0, 0, 0, 0],  # qp(t1, n1)
                    (1, 1): [1, 1, 0, 0, 0, 0, 0, 0, 0],  # qp(t2/t3, n1)
                    (1, 2): [1, 1, 1, 0, 0, 0, 0, 0, 0],  # ffn(0, 0/1/2)
                    (1, 3): [1, 0, 0, 0, 0, 0, 0, 0, 0],  # ffn(0, 3)
                }
                # flat pipeline over all (n, t, m): each slot emits the NEXT
                # unit's scores+exp BEFORE the previous unit's ctx, so a new
                # pair's scores sit ahead of the old pair's last ctx in the PE
                # stream and the ACT never waits out a pair boundary.
                pairs = [(n, t) for n in range(NQ) for t in range(MT)]
                flat_pops = []
                for p in pairs:
                    flat_pops.extend(scheds[p][:KT8])
                flat_pops.append(0)
                units = [(n, t, m) for (n, t) in pairs for m in range(KT8)]
                exq = []
                pc_cur = [None]
                for sl in range(len(units) + 1):
                    if sl < len(units):
                        n, t, m = units[sl]
                        nsl = slice(n * 512, (n + 1) * 512)
                        ps = spool.tile([128, 2, 512], F32, name="ps", tag="ps")
                        nc.tensor.matmul(
                            ps[:, 0, :],
                            kpb[0:64, t, m * 128:(m + 1) * 128],
                            qp[0:64, t, nsl], start=True, stop=True,
                        )
                        nc.tensor.matmul(
                            ps[:, 1, :],
                            kpb[64:128, t, m * 128:(m + 1) * 128],
                            qp[64:128, t, nsl], start=True, stop=True,
                        )
                        ex = epool.tile([128, 2, 512], BF16, name="ex", tag="ex")
                        nc.scalar.activation(
                            ex[:, :, :], ps[:, :, :], ACTF.Exp, scale=scale
                        )
                        if dbg and sl == 0:
                            nc.sync.dma_start(out=dDEX[:, :, :], in_=ex[:, :, :])
                        exq.append(ex)
                    if sl >= 1:
                        n0, t0, m0 = units[sl - 1]
                        ex = exq.pop(0)
                        if m0 == 0:
                            pc_cur[0] = cpool.tile(
                                [DH + 1, 2, 512], F32, name="pc", tag="pc"
                            )
                        pc = pc_cur[0]
                        for hh in range(2):
                            nc.tensor.matmul(
                                pc[:, hh, :],
                                vpa[:, m0, 2 * t0 + hh, :],
                                ex[:, hh, :],
                                start=(m0 == 0), stop=(m0 == KT8 - 1),
                            )
                        if m0 == KT8 - 1:
                            normalize(n0, t0, pc)
                            if (n0, t0) == (0, MT - 1):
                                # FFN for n=0 becomes filler work during n=1
                                for mo in range(MT):
                                    fillers.append(lambda mo=mo: ffn(0, mo))
                    for _ in range(flat_pops[sl]):
                        if fillers:
                            fillers.pop(0)()
                while fillers:
                    fillers.pop(0)()
                for mo in range(MT):
                    ffn(1, mo)

                if dbg:
                    nc.sync.dma_start(out=dDQP[:, :, :], in_=qp[:, :, :])
                    nc.sync.dma_start(out=dDKP[:, :, :], in_=kpb[:, :, :])
                    nc.sync.dma_start(out=dDO1[:, :, :], in_=o1[:, :, :])

    nc.compile()
    return nc


def _get_nc():
    global _NC
    if _NC is None:
        _NC = _build()
    return _NC


def _prep_inputs(Q, K, Wq, bq, Wk, bk, Wv, bv, Wo, bo):
    import ml_dtypes
    bf = ml_dtypes.bfloat16

    Q = np.asarray(Q, dtype=np.float32)
    K = np.asarray(K, dtype=np.float32)
    Wq = np.asarray(Wq, dtype=np.float32)
    Wk = np.asarray(Wk, dtype=np.float32)
    Wv = np.asarray(Wv, dtype=np.float32)
    Wo = np.asarray(Wo, dtype=np.float32)
    bq = np.asarray(bq, dtype=np.float32)
    bk = np.asarray(bk, dtype=np.float32)
    bv = np.asarray(bv, dtype=np.float32)
    bo = np.asarray(bo, dtype=np.float32)

    bo2 = (bo + Wo @ bv).astype(np.float32)

    def btile(b):
        return np.ascontiguousarray(b.reshape(MT, 128).T)

    def pack(wT, free):
        # [din, free] -> SBUF layout [128, KC, free]
        return np.ascontiguousarray(
            wT.reshape(KC, 128, free).transpose(1, 0, 2)
        ).astype(bf)

    def pack2(aT, free):
        # [din, free] -> half-major SBUF layout [128, 2, KC, free//2]
        a = aT.reshape(KC, 128, 2, free // 2)
        return np.ascontiguousarray(a.transpose(1, 2, 0, 3)).astype(bf)

    shared = {
        "WqT": pack(Wq.T, D),
        "WkT": pack(Wk.T, D),
        "WvT": pack(Wv.T, D),
        "WoT": pack(Wo.T, D),
        "BQ": btile(bq),
        "BK": btile(bk),
        "BO2": btile(bo2),
        "BV": btile(bv),
    }
    in_maps = []
    for c in range(N_CORES):
        m = dict(shared)
        m["QT"] = pack2(Q[c].T, SQ)
        m["KT"] = pack2(K[c].T, SK)
        in_maps.append(m)
    return in_maps


def run(inputs, trace=False):
    """Run on hardware; returns (output [B,SQ,D] f32, BassKernelResults)."""
    in_maps = _prep_inputs(
        inputs["Q"], inputs["K"], inputs["Wq"], inputs["bq"], inputs["Wk"],
        inputs["bk"], inputs["Wv"], inputs["bv"], inputs["Wo"], inputs["bo"],
    )
    nc = _get_nc()
    res = run_bass_kernel_spmd(
        nc, in_maps, core_ids=list(range(N_CORES)), trace=trace
    )
    out = np.stack(
        [res.results[c]["OT"].T for c in range(N_CORES)], axis=0
    ).astype(np.float32)
    return out, res


def kernel(**inputs):
    nh = inputs.get("num_heads", H)
    assert int(nh) == H, f"kernel hardcodes num_heads={H}, got {nh}"
    out, _ = run(inputs, trace=False)
    return out


if __name__ == "__main__":
    rng = np.random.default_rng(0)
    inputs = {
        "Q": rng.standard_normal((B, SQ, D), dtype=np.float32),
        "K": rng.standard_normal((B, SK, D), dtype=np.float32),
        "Wq": rng.standard_normal((D, D), dtype=np.float32) * 0.04,
        "bq": rng.standard_normal((D,), dtype=np.float32) * 0.04,
        "Wk": rng.standard_normal((D, D), dtype=np.float32) * 0.04,
        "bk": rng.standard_normal((D,), dtype=np.float32) * 0.04,
        "Wv": rng.standard_normal((D, D), dtype=np.float32) * 0.04,
        "bv": rng.standard_normal((D,), dtype=np.float32) * 0.04,
        "Wo": rng.standard_normal((D, D), dtype=np.float32) * 0.04,
        "bo": rng.standard_normal((D,), dtype=np.float32) * 0.04,
        "num_heads": H,
    }
    out = kernel(**inputs)
    print("out", out.shape, out.dtype, float(np.abs(out).max()))


# revision 6
# speedup vs baseline: 1.1115x; 1.0484x over previous
"""Trainium2 Bass kernel for nn_MAB_44057774522768 (Set-Transformer MAB block), v2.

Per batch b (B=8 -> core b), Sq=Sk=1024, D=512, H=8 heads, dh=64:
    Qp = Q @ Wq.T + bq;  Kp = K @ Wk.T + bk;  Vp = K @ Wv.T (bv folded out)
    scores_h = Qp_h @ Kp_h.T / sqrt(D);  A = softmax(scores)
    ctx_h = A_h @ Vp_h;  O1 = Qp + ctx
    out = O1 + bv + relu(O1 @ Wo.T + bo + Wo@bv)

v2 design (vs v1 baseline at ~180us/core):
  * all-bf16 inputs/weights (halves HBM traffic; f32 only in PSUM + final out)
  * scores for a head pair run CONCURRENTLY as K=64 row-tiles (partitions
    0-63 / 64-127) -> 2x on the score matmuls
  * scores output bf16 directly to PSUM (1KB/bank), exp'd in one ACT op per
    (pair, k-tile) of FD=1024 -> ACT (the 73us exp floor) stays saturated
  * q-chunked dataflow (n = 512-col chunk outer loop): ctx accumulators are
    [65, 512] -> whole pipeline fits in 8 PSUM banks with double-buffering,
    and FFN(n=0) + its output DMA overlap n=1's attention
  * softmax denominator via ones-column in Vp (row 64 of ctx PSUM);
    whole-tile fp32 reciprocal (single-partition DVE slices at base partition
    64 silently execute at partition 0 -- never slice-recip row 64 alone),
    then DRAM-bounce + partition-step-0 broadcast DMA back to 64 partitions
  * projections and FFN drip-fed into attention's PE gaps as fillers
"""

import math
import os

import numpy as np

import concourse.bass as bass
import concourse.mybir as mybir
import concourse.tile as tile
from concourse import bacc
from concourse.bass_utils import run_bass_kernel_spmd

B, SQ, SK, D = 8, 1024, 1024, 512
H, DH = 8, 64
N_CORES = 8
KC = D // 128   # 4 contraction chunks of 128 (din)
MT = D // 128   # 4 output-feature tiles of 128 (dout)
NQ = SQ // 512  # 2 q chunks of 512
KT8 = SK // 128 # 8 key-seq tiles of 128

F32 = mybir.dt.float32
BF16 = mybir.dt.bfloat16
ALU = mybir.AluOpType
ACTF = mybir.ActivationFunctionType

_NC = None


def _build():
    nc = bacc.Bacc(None, target_bir_lowering=False, debug=False)

    # inputs host-packed to the exact SBUF layout [partition, kc, free] so
    # each loads in ONE descriptor with 4-8KB partition lines (the 3 HWDGE
    # queues run ~20GB/s on 1KB lines but much faster on big contiguous ones)
    dQT = nc.dram_tensor("QT", [128, 2, KC, SQ // 2], BF16, kind="ExternalInput")
    dKT = nc.dram_tensor("KT", [128, 2, KC, SK // 2], BF16, kind="ExternalInput")
    dWq = nc.dram_tensor("WqT", [128, KC, D], BF16, kind="ExternalInput")
    dWk = nc.dram_tensor("WkT", [128, KC, D], BF16, kind="ExternalInput")
    dWv = nc.dram_tensor("WvT", [128, KC, D], BF16, kind="ExternalInput")
    dWo = nc.dram_tensor("WoT", [128, KC, D], BF16, kind="ExternalInput")
    dBQ = nc.dram_tensor("BQ", [128, MT], F32, kind="ExternalInput")
    dBK = nc.dram_tensor("BK", [128, MT], F32, kind="ExternalInput")
    dBO2 = nc.dram_tensor("BO2", [128, MT], F32, kind="ExternalInput")
    dBV = nc.dram_tensor("BV", [128, MT], F32, kind="ExternalInput")
    dOT = nc.dram_tensor("OT", [D, SQ], F32, kind="ExternalOutput")

    dbg = os.environ.get("KDEBUG", "0") == "1"
    if dbg:
        dDQP = nc.dram_tensor("DQP", [128, MT, SQ], BF16, kind="ExternalOutput")
        dDKP = nc.dram_tensor("DKP", [128, MT, SK], BF16, kind="ExternalOutput")
        dDEX = nc.dram_tensor("DEX", [128, 2, 512], BF16, kind="ExternalOutput")
        dDO1 = nc.dram_tensor("DO1", [128, MT, SQ], BF16, kind="ExternalOutput")
        dDCU = nc.dram_tensor("DCU", [128, 512], F32, kind="ExternalOutput")
        dDRB = nc.dram_tensor("DRB", [128, 512], F32, kind="ExternalOutput")

    scale = 1.0 / math.sqrt(float(D))
    ctx_win = int(os.environ.get("KCTXWIN", "0"))

    with tile.TileContext(nc) as tc:
        with (
            tc.tile_pool(name="persist", bufs=1) as persist,
            tc.tile_pool(name="ppool", bufs=2, space="PSUM") as ppool,   # 1 bank x2
            tc.tile_pool(name="spool", bufs=2, space="PSUM") as spool,   # 2 banks x2
            tc.tile_pool(name="cpool", bufs=1, space="PSUM") as cpool,   # 2 banks
            tc.tile_pool(name="epool", bufs=max(3, ctx_win + 2)) as epool,
            tc.tile_pool(name="zpool", bufs=2) as zpool,
            tc.tile_pool(name="ypool", bufs=2) as ypool,
            tc.tile_pool(name="rpool", bufs=2) as rpool,
            tc.tile_pool(name="rbpool", bufs=2) as rbpool,
            tc.tile_pool(name="dpool", bufs=2, space="DRAM") as dpool,
            tc.tile_pool(name="cnpool", bufs=2) as cnpool,
            tc.tile_pool(name="fpool", bufs=2) as fpool,
            tc.tile_pool(name="outpool", bufs=3) as outpool,
        ):
            # ---- persistent SBUF tensors ----
            qt = persist.tile([128, 2, KC, SQ // 2], BF16)  # half-major
            kt = persist.tile([128, 2, KC, SK // 2], BF16)
            wq = persist.tile([128, KC, D], BF16)
            wk = persist.tile([128, KC, D], BF16)
            wv = persist.tile([128, KC, D], BF16)
            wo = persist.tile([128, KC, D], BF16)
            bq = persist.tile([128, MT], F32)
            bk = persist.tile([128, MT], F32)
            bo2 = persist.tile([128, MT], F32)
            bv = persist.tile([128, MT], F32)
            qp = persist.tile([128, MT, SQ], BF16)   # projected Q (scores rhs + residual)
            kpb = persist.tile([128, MT, SK], BF16)
            # Vp seq-major [k, kt8, h, dh] + ones column at dh=64 per head
            vpa = persist.tile([128, KT8, H, DH + 1], BF16)
            o1 = persist.tile([128, MT, SQ], BF16)

            # ---- input DMAs: one big descriptor per tensor, spread across
            # the 3 queues by first-need (wq/wk gate the upfront projections,
            # qt/kt gate scores, wv gates ctx, wo is needed only by the FFN)
            # sync (SP) is the slow software queue (~50GB/s) -- biases only.
            # scalar + gpsimd are fast hardware queues (~150-250GB/s): weights
            # first (they gate the upfront projections), then activations.
            nc.scalar.dma_start(out=wq[:, :, :], in_=dWq[:, :, :])
            nc.gpsimd.dma_start(out=wk[:, :, :], in_=dWk[:, :, :])
            nc.scalar.dma_start(out=qt[:, 0, :, :], in_=dQT[:, 0, :, :])
            nc.gpsimd.dma_start(out=kt[:, 0, :, :], in_=dKT[:, 0, :, :])
            nc.scalar.dma_start(out=wv[:, :, :], in_=dWv[:, :, :])
            nc.gpsimd.dma_start(out=kt[:, 1, :, :], in_=dKT[:, 1, :, :])
            nc.scalar.dma_start(out=qt[:, 1, :, :], in_=dQT[:, 1, :, :])
            nc.gpsimd.dma_start(out=wo[:, :, :], in_=dWo[:, :, :])
            nc.sync.dma_start(out=bq, in_=dBQ[:, :])
            nc.sync.dma_start(out=bk, in_=dBK[:, :])
            nc.sync.dma_start(out=bo2, in_=dBO2[:, :])
            nc.sync.dma_start(out=bv, in_=dBV[:, :])

            # ones column for the fused softmax denominator
            nc.vector.memset(vpa[:, :, :, DH:DH + 1], 1.0)

            def project(dst, w, rhs_src, bias_ap, t, n):
                """dst[:, t, nsl] = (w[:,:,t-tile].T @ rhs_src[:,:,nsl]) + bias, bf16."""
                nsl = slice(n * 512, (n + 1) * 512)
                pp = ppool.tile([128, 512], F32, name="pp", tag="pp")
                for kc in range(KC):
                    nc.tensor.matmul(
                        pp[:, :],
                        w[:, kc, t * 128:(t + 1) * 128],
                        rhs_src[:, n, kc, :],
                        start=(kc == 0),
                        stop=(kc == KC - 1),
                    )
                nc.vector.tensor_scalar(dst[:, t, nsl], pp[:, :], bias_ap, None, ALU.add)

            def project_v(mt):
                """vpa[:, mt, :, 0:64] = (kt[:,:,mt-tile].T @ wv) in bf16."""
                pv = ppool.tile([128, 512], F32, name="pv", tag="pp")
                mh, mo_ = mt // 4, mt % 4
                for kc in range(KC):
                    nc.tensor.matmul(
                        pv[:, :],
                        kt[:, mh, kc, mo_ * 128:(mo_ + 1) * 128],
                        wv[:, kc, :],
                        start=(kc == 0),
                        stop=(kc == KC - 1),
                    )
                nc.vector.tensor_copy(
                    vpa[:, mt, :, 0:DH],
                    pv[:, :].rearrange("p (h d) -> p h d", h=H),
                )

            def ffn(n, mo):
                """dOT[mo-tile, nsl] = o1 + bv + relu(wo.T @ o1 + bo2)."""
                nsl = slice(n * 512, (n + 1) * 512)
                pf = ppool.tile([128, 512], F32, name="pf", tag="pp")
                for kc in range(KC):
                    nc.tensor.matmul(
                        pf[:, :],
                        wo[:, kc, mo * 128:(mo + 1) * 128],
                        o1[:, kc, nsl],
                        start=(kc == 0),
                        stop=(kc == KC - 1),
                    )
                rf = fpool.tile([128, 512], F32, name="rf", tag="rf")
                nc.vector.tensor_scalar(
                    rf[:, :], pf[:, :], bo2[:, mo:mo + 1], 0.0, ALU.add, ALU.max
                )
                ot = outpool.tile([128, 512], F32, name="ot", tag="ot")
                nc.vector.scalar_tensor_tensor(
                    ot[:, :], rf[:, :], bv[:, mo:mo + 1], o1[:, mo, nsl],
                    ALU.add, ALU.add,
                )
                rowsl = slice(mo * 128, (mo + 1) * 128)
                if n == 0:
                    nc.gpsimd.dma_start(out=dOT[rowsl, 0:256], in_=ot[:, 0:256])
                    nc.sync.dma_start(out=dOT[rowsl, 256:512], in_=ot[:, 256:512])
                else:
                    nc.gpsimd.dma_start(out=dOT[rowsl, 512:768], in_=ot[:, 0:256])
                    nc.scalar.dma_start(out=dOT[rowsl, 768:896], in_=ot[:, 256:384])
                    nc.sync.dma_start(out=dOT[rowsl, 896:1024], in_=ot[:, 384:512])

            # deferred work units, drip-fed into attention PE gaps
            fillers = []

            def attend_pair(n, t, fill_sched):
                """Heads 2t (scores rows 0-63) and 2t+1 (rows 64-127), q-chunk n.

                fill_sched: pops-per-iteration list (len KT8+1); fillers a pair's
                scores depend on must be popped in an EARLIER pair (in-order
                engine programs), except project_v(m) which may pop at iter <= m
                of pair (0,0) since ctx(m) is emitted at iter m+1.
                """
                nsl = slice(n * 512, (n + 1) * 512)
                pc = cpool.tile([DH + 1, 2, 512], F32, name="pc", tag="pc")
                # ctx window: accumulate ctx in contiguous runs of W k-tiles per
                # head so the A/B PSUM accumulation groups never interleave
                # (interleaved groups corrupt PSUM on HW; cf. v1 docstring)
                W = int(os.environ.get("KCTXWIN", "0"))
                ps_q = []   # software pipeline: scores ahead of ctx
                for m in range(KT8 + 1):
                    npop = fill_sched[m] if m < len(fill_sched) else 0
                    if m < KT8:
                        ps = spool.tile([128, 2, 512], F32, name="ps", tag="ps")
                        nc.tensor.matmul(
                            ps[:, 0, :],
                            kpb[0:64, t, m * 128:(m + 1) * 128],
                            qp[0:64, t, nsl],
                            start=True, stop=True,
                        )
                        nc.tensor.matmul(
                            ps[:, 1, :],
                            kpb[64:128, t, m * 128:(m + 1) * 128],
                            qp[64:128, t, nsl],
                            start=True, stop=True,
                        )
                        ex = epool.tile([128, 2, 512], BF16, name="ex", tag="ex")
                        nc.scalar.activation(
                            ex[:, :, :], ps[:, :, :], ACTF.Exp, scale=scale
                        )
                        if dbg and n == 0 and t == 0 and m == 0:
                            nc.sync.dma_start(out=dDEX[:, :, :], in_=ex[:, :, :])
                        ps_q.append((m, ex))
                    if W == 0:
                        if m >= 1:
                            mm, ex = ps_q.pop(0)
                            for hh in range(2):
                                nc.tensor.matmul(
                                    pc[:, hh, :],
                                    vpa[:, mm, 2 * t + hh, :],
                                    ex[:, hh, :],
                                    start=(mm == 0),
                                    stop=(mm == KT8 - 1),
                                )
                    elif m < KT8 and (m + 1) % W == 0:
                        batch, ps_q = ps_q, []
                        for hh in range(2):
                            for mm, ex in batch:
                                nc.tensor.matmul(
                                    pc[:, hh, :],
                                    vpa[:, mm, 2 * t + hh, :],
                                    ex[:, hh, :],
                                    start=(mm == 0),
                                    stop=(mm == KT8 - 1),
                                )
                    for _ in range(npop):
                        if fillers:
                            fillers.pop(0)()

                normalize(n, t, pc)

            def normalize(n, t, pc):
                nsl = slice(n * 512, (n + 1) * 512)
                # ---- normalize + residual for this (n, t) ----
                # Z rows 0-63: ctx_A, row 64: denom_A (later rows 64-127 <- ctx_B)
                Z = zpool.tile([128, 512], F32, name="Z", tag="Z")
                Y = ypool.tile([DH + 1, 512], F32, name="Y", tag="Y")
                nc.vector.tensor_copy(Z[0:DH + 1, :], pc[:, 0, :])
                nc.vector.tensor_copy(Y[:, :], pc[:, 1, :])
                # whole-tile reciprocals (single-partition DVE slices at a
                # nonzero base partition misexecute at partition 0 -- only the
                # row-64 result is used)
                R = rpool.tile([DH + 1, 1024], F32, name="R", tag="R")
                nc.vector.reciprocal_approx_fast(R[:, 0:512], Z[0:DH + 1, :])
                nc.vector.reciprocal_approx_fast(R[:, 512:1024], Y[:, :])
                # DRAM-bounce the recip denom row, broadcast back to 64
                # partitions each (partition-step-0 APs are DRAM-only)
                rec_d = dpool.tile([1, 1024], F32, name="rec_d", tag="rec_d")
                # the last pair's bounce rides the scalar hardware queue (idle
                # once the final exp is done, and much faster than sync/SP)
                beng = nc.scalar if (n, t) == (NQ - 1, MT - 1) else nc.sync
                beng.dma_start(out=rec_d[:, :], in_=R[DH:DH + 1, :])
                rb = rbpool.tile([128, 512], F32, name="rb", tag="rb")
                srcA = bass.AP(
                    tensor=rec_d[0:1, 0:512].tensor,
                    offset=rec_d[0:1, 0:512].offset,
                    ap=[[0, 64], [1, 512]],
                )
                srcB = bass.AP(
                    tensor=rec_d[0:1, 512:1024].tensor,
                    offset=rec_d[0:1, 512:1024].offset,
                    ap=[[0, 64], [1, 512]],
                )
                beng.dma_start(out=rb[0:64, :], in_=srcA)
                beng.dma_start(out=rb[64:128, :], in_=srcB)
                # shift ctx_B into partitions 64-127 (overwrites denom_A row too)
                nc.gpsimd.dma_start(out=Z[64:128, :], in_=Y[0:DH, :])
                cn = cnpool.tile([128, 512], BF16, name="cn", tag="cn")
                nc.vector.tensor_mul(cn[:, :], Z[:, :], rb[:, :])
                nc.vector.tensor_add(o1[:, t, nsl], cn[:, :], qp[:, t, nsl])
                if dbg and n == 0 and t == 0:
                    nc.sync.dma_start(out=dDCU[:, :], in_=Z[:, :])
                    nc.sync.dma_start(out=dDRB[:, :], in_=rb[:, :])

            # ---- emission ----
            # minimal upfront: pair (0,0) prerequisites (kpb keys 512+ are
            # needed only from k-tile m=4 -> kp(t0,k1) is filler iter 0)
            project(qp, wq, qt, bq[:, 0:1], 0, 0)
            project(kpb, wk, kt, bk[:, 0:1], 0, 0)

            if os.environ.get("KNOFILL", "0") == "1":
                # bisect mode: no filler interleaving; everything upfront
                for t in range(MT):
                    for nn in range(NQ):
                        if (t, nn) != (0, 0):
                            project(qp, wq, qt, bq[:, t:t + 1], t, nn)
                            project(kpb, wk, kt, bk[:, t:t + 1], t, nn)
                for mt in range(KT8):
                    project_v(mt)
                for n in range(NQ):
                    for t in range(MT):
                        attend_pair(n, t, [0] * (KT8 + 1))
                for n in range(NQ):
                    for mo in range(MT):
                        ffn(n, mo)
                if dbg:
                    nc.sync.dma_start(out=dDQP[:, :, :], in_=qp[:, :, :])
                    nc.sync.dma_start(out=dDKP[:, :, :], in_=kpb[:, :, :])
                    nc.sync.dma_start(out=dDO1[:, :, :], in_=o1[:, :, :])
                emit_main = False
            else:
                emit_main = True

            # fillers, in dependency order. Pair (0,0) interleaves the t=1/n=0
            # projections between the project_v units (pv(m) must pop by iter m
            # of pair (0,0); the t1 projections by end of pair (0,0)); every
            # other unit is popped at least one pair before its consumer.
            if emit_main:
                # kpb is indexed by KEY chunk (both halves needed by iter 4 of
                # ANY pair of that t); qp by q-chunk n. pv(m) must pop by iter
                # m of pair (0,0); all else pops >=1 pair before its consumer.
                fillers.append(lambda: project(kpb, wk, kt, bk[:, 0:1], 0, 1))
                fillers.append(lambda: project_v(0))
                fillers.append(lambda: project(kpb, wk, kt, bk[:, 1:2], 1, 0))
                fillers.append(lambda: project_v(1))
                fillers.append(lambda: project(kpb, wk, kt, bk[:, 1:2], 1, 1))
                fillers.append(lambda: project_v(2))
                fillers.append(lambda: project(qp, wq, qt, bq[:, 1:2], 1, 0))
                for mt in range(3, KT8):
                    fillers.append(lambda mt=mt: project_v(mt))
                for t in range(2, MT):
                    fillers.append(lambda t=t: project(kpb, wk, kt, bk[:, t:t + 1], t, 0))
                    fillers.append(lambda t=t: project(kpb, wk, kt, bk[:, t:t + 1], t, 1))
                    fillers.append(lambda t=t: project(qp, wq, qt, bq[:, t:t + 1], t, 0))
                for t in range(MT):
                    fillers.append(lambda t=t: project(qp, wq, qt, bq[:, t:t + 1], t, 1))

                scheds = {
                    (0, 0): [2, 2, 2, 2, 1, 1, 1, 1, 0],  # kp(t0,k1), 8 pv, t1 set
                    (0, 1): [1, 1, 1, 0, 0, 0, 0, 0, 0],  # t2 set
                    (0, 2): [1, 1, 1, 0, 0, 0, 0, 0, 0],  # t3 set
                    (0, 3): [1, 0, 0, 0, 0, 0, 0, 0, 0],  # qp(t0, n1)
                    (1, 0): [1, 0, 0, 0, 0, 0, 0, 0, 0],  # qp(t1, n1)
                    (1, 1): [1, 1, 0, 0, 0, 0, 0, 0, 0],  # qp(t2/t3, n1)
                    (1, 2): [1, 1, 1, 0, 0, 0, 0, 0, 0],  # ffn(0, 0/1/2)
                    (1, 3): [1, 0, 0, 0, 0, 0, 0, 0, 0],  # ffn(0, 3)
                }
                # flat pipeline over all (n, t, m): each slot emits the NEXT
                # unit's scores+exp BEFORE the previous unit's ctx, so a new
                # pair's scores sit ahead of the old pair's last ctx in the PE
                # stream and the ACT never waits out a pair boundary.
                pairs = [(n, t) for n in range(NQ) for t in range(MT)]
                flat_pops = []
                for p in pairs:
                    flat_pops.extend(scheds[p][:KT8])
                flat_pops.append(0)
                units = [(n, t, m) for (n, t) in pairs for m in range(KT8)]
                exq = []
                pc_cur = [None]
                for sl in range(len(units) + 1):
                    if sl < len(units):
                        n, t, m = units[sl]
                        nsl = slice(n * 512, (n + 1) * 512)
                        ps = spool.tile([128, 2, 512], F32, name="ps", tag="ps")
                        nc.tensor.matmul(
                            ps[:, 0, :],
                            kpb[0:64, t, m * 128:(m + 1) * 128],
                            qp[0:64, t, nsl], start=True, stop=True,
                        )
                        nc.tensor.matmul(
                            ps[:, 1, :],
                            kpb[64:128, t, m * 128:(m + 1) * 128],
                            qp[64:128, t, nsl], start=True, stop=True,
                        )
                        ex = epool.tile([128, 2, 512], BF16, name="ex", tag="ex")
                        nc.scalar.activation(
                            ex[:, :, :], ps[:, :, :], ACTF.Exp, scale=scale
                        )
                        if dbg and sl == 0:
                            nc.sync.dma_start(out=dDEX[:, :, :], in_=ex[:, :, :])
                        exq.append(ex)
                    if sl >= 1:
                        n0, t0, m0 = units[sl - 1]
                        ex = exq.pop(0)
                        if m0 == 0:
                            pc_cur[0] = cpool.tile(
                                [DH + 1, 2, 512], F32, name="pc", tag="pc"
                            )
                        pc = pc_cur[0]
                        for hh in range(2):
                            nc.tensor.matmul(
                                pc[:, hh, :],
                                vpa[:, m0, 2 * t0 + hh, :],
                                ex[:, hh, :],
                                start=(m0 == 0), stop=(m0 == KT8 - 1),
                            )
                        if m0 == KT8 - 1:
                            normalize(n0, t0, pc)
                            if (n0, t0) == (0, MT - 1):
                                # FFN for n=0 becomes filler work during n=1
                                for mo in range(MT):
                                    fillers.append(lambda mo=mo: ffn(0, mo))
                    for _ in range(flat_pops[sl]):
                        if fillers:
                            fillers.pop(0)()
                while fillers:
                    fillers.pop(0)()
                for mo in range(MT):
                    ffn(1, mo)

                if dbg:
                    nc.sync.dma_start(out=dDQP[:, :, :], in_=qp[:, :, :])
                    nc.sync.dma_start(out=dDKP[:, :, :], in_=kpb[:, :, :])
                    nc.sync.dma_start(out=dDO1[:, :, :], in_=o1[:, :, :])

    nc.compile()
    return nc


def _get_nc():
    global _NC
    if _NC is None:
        _NC = _build()
    return _NC


def _prep_inputs(Q, K, Wq, bq, Wk, bk, Wv, bv, Wo, bo):
    import ml_dtypes
    bf = ml_dtypes.bfloat16

    Q = np.asarray(Q, dtype=np.float32)
    K = np.asarray(K, dtype=np.float32)
    Wq = np.asarray(Wq, dtype=np.float32)
    Wk = np.asarray(Wk, dtype=np.float32)
    Wv = np.asarray(Wv, dtype=np.float32)
    Wo = np.asarray(Wo, dtype=np.float32)
    bq = np.asarray(bq, dtype=np.float32)
    bk = np.asarray(bk, dtype=np.float32)
    bv = np.asarray(bv, dtype=np.float32)
    bo = np.asarray(bo, dtype=np.float32)

    bo2 = (bo + Wo @ bv).astype(np.float32)

    def btile(b):
        return np.ascontiguousarray(b.reshape(MT, 128).T)

    def pack(wT, free):
        # [din, free] -> SBUF layout [128, KC, free]
        return np.ascontiguousarray(
            wT.reshape(KC, 128, free).transpose(1, 0, 2)
        ).astype(bf)

    def pack2(aT, free):
        # [din, free] -> half-major SBUF layout [128, 2, KC, free//2]
        a = aT.reshape(KC, 128, 2, free // 2)
        return np.ascontiguousarray(a.transpose(1, 2, 0, 3)).astype(bf)

    shared = {
        "WqT": pack(Wq.T, D),
        "WkT": pack(Wk.T, D),
        "WvT": pack(Wv.T, D),
        "WoT": pack(Wo.T, D),
        "BQ": btile(bq),
        "BK": btile(bk),
        "BO2": btile(bo2),
        "BV": btile(bv),
    }
    in_maps = []
    for c in range(N_CORES):
        m = dict(shared)
        m["QT"] = pack2(Q[c].T, SQ)
        m["KT"] = pack2(K[c].T, SK)
        in_maps.append(m)
    return in_maps


def run(inputs, trace=False):
    """Run on hardware; returns (output [B,SQ,D] f32, BassKernelResults)."""
    in_maps = _prep_inputs(
        inputs["Q"], inputs["K"], inputs["Wq"], inputs["bq"], inputs["Wk"],
        inputs["bk"], inputs["Wv"], inputs["bv"], inputs["Wo"], inputs["bo"],
    )
    nc = _get_nc()
    res = run_bass_kernel_spmd(
        nc, in_maps, core_ids=list(range(N_CORES)), trace=trace
    )
    out = np.stack(
        [res.results[c]["OT"].T for c in range(N_CORES)], axis=0
    ).astype(np.float32)
    return out, res


def kernel(**inputs):
    nh = inputs.get("num_heads", H)
    assert int(nh) == H, f"kernel hardcodes num_heads={H}, got {nh}"
    out, _ = run(inputs, trace=False)
    return out


if __name__ == "__main__":
    rng = np.random.default_rng(0)
    inputs = {
        "Q": rng.standard_normal((B, SQ, D), dtype=np.float32),
        "K": rng.standard_normal((B, SK, D), dtype=np.float32),
        "Wq": rng.standard_normal((D, D), dtype=np.float32) * 0.04,
        "bq": rng.standard_normal((D,), dtype=np.float32) * 0.04,
        "Wk": rng.standard_normal((D, D), dtype=np.float32) * 0.04,
        "bk": rng.standard_normal((D,), dtype=np.float32) * 0.04,
        "Wv": rng.standard_normal((D, D), dtype=np.float32) * 0.04,
        "bv": rng.standard_normal((D,), dtype=np.float32) * 0.04,
        "Wo": rng.standard_normal((D, D), dtype=np.float32) * 0.04,
        "bo": rng.standard_normal((D,), dtype=np.float32) * 0.04,
        "num_heads": H,
    }
    out = kernel(**inputs)
    print("out", out.shape, out.dtype, float(np.abs(out).max()))
